# revision 28
# baseline (speedup 1.0000x reference)
"""Trainium2 Bass kernel for nn_MIGAModel (moe_routing).

Strategy (pure data parallel over the stock axis N, 8 cores):
 - The x stream (94.8MB/core, ~264us at 360GB/s) is the roofline;
   everything else hides under it.  x ships as bf16 (hi, lo) plane
   pairs (same bytes as fp32, exact to ~2^-18), pre-transposed and
   column-blocked on host so each block is contiguous in DRAM.
 - Router h = Whi.xhi + Wlo.xhi + Whi.xlo: three bf16 matmuls per
   K-tile at 1 PE cycle/row (vs 4 for fp32) accumulate in fp32 PSUM.
   h error ~3e-6 -> zero top-2 flips (gating ranking is the
   precision-critical part: plain bf16/fp16 routing fails the 2e-2
   gate; plain fp32 matmuls would make PE the bottleneck; fp32r needs
   pre-rounded inputs on real HW, i.e. is not fp32-accurate).
 - Column-block pipeline: while block c+1's x streams, block c is
   post-processed; trailing blocks are narrower so the only exposed
   post-chain (the last block's) is short.
 - Host algebra folds the expert layer into Q/K/V (eo never exists),
   all q/k biases into one score matmul on h (MSCH) + a constant in
   the exp-activation bias, and the v biases into one matmul on the
   score numerators (MBV).  The value path runs in fp16 (error
   ~1e-3, gating unaffected; HW rel err ~1.25e-2 is dominated by a
   couple of top-2 near-tie flips from the router's ~3e-6 h-noise).
 - Gating and the final weighted combine use Pool-engine
   partition_all_reduce (max/add across the 128 expert partitions):
   no transposes or broadcast matmuls; the reduces overlap the DVE
   compare/select and value-path ops (real HW restricts Pool to the
   reduce family, DVE to one PSUM operand per op, and has no DVE
   divide -- hence the ACT-engine drains and recip+mul).
"""
import sys
import numpy as np
import ml_dtypes

for _p in ("/opt/trn_rl_repo",):
    if _p not in sys.path:
        sys.path.insert(0, _p)

import concourse.bass as bass
import concourse.tile as tile
from concourse import bacc, mybir, bass_isa
from concourse.bass_utils import run_bass_kernel_spmd

F32 = mybir.dt.float32
BF16 = mybir.dt.bfloat16
F16 = mybir.dt.float16
NPBF = ml_dtypes.bfloat16

N, T, D = 20000, 60, 158
TD = T * D                      # 9480
G, E, H, DH, GE = 8, 16, 4, 4, 128
NCORES = 8
NSH = N // NCORES               # 2500 rows per core
KT = TD // 128                  # 74 full K-tiles
KREM = TD - KT * 128            # 8 remainder contraction rows
TDP = (KT + 1) * 128            # wr ships padded to 75 tiles

# column blocks: sum = NSH; >=256 keeps DMA lines >=1KB; narrow
# trailing blocks shorten the exposed final post-chain
WIDTHS = [512, 512, 512, 452, 256, 128, 128]
assert sum(WIDTHS) == NSH and all(128 <= w <= 512 for w in WIDTHS)
WMAX = max(WIDTHS)
OFFS = [sum(WIDTHS[:i]) for i in range(len(WIDTHS))]


def ksubs_for(w, last=False):
    """K-tiles per sub-DMA over the 74 full tiles: ~10KB SBUF tiles.
    The final block tapers so almost no accumulation trails the stream."""
    nk = max(1, 2560 // w)
    ks = [nk] * (KT // nk)
    if KT % nk:
        ks.append(KT % nk)
    if last and ks[-1] > 4:
        ks[-1:] = [ks[-1] - 4, 4]
    return ks


# packed matrix indices (each a [128,128] block in the "mats" input)
M_AQ = 0
M_AK0, M_AV0 = 1, 5             # 4 each
M_MS0 = 9                       # 4
M_MSCH = 13
M_MDEN = 14
M_MBV = 15
M_MER0 = 16                     # 4
M_AO = 20
NMATS = 21

# bias pack columns
B_BO, B_BR, B_SC = 0, 1, 2
NBIAS = 4


def bf16pair(a):
    """Split fp32 array into (hi, lo) bf16 planes, hi+lo ~= a to 2^-18."""
    a = np.asarray(a, np.float32)
    hi = a.astype(NPBF)
    lo = (a - hi.astype(np.float32)).astype(NPBF)
    return hi, lo


def build_consts(Wr, br, We, be, Wq, bq, Wk, bk, Wv, bv, Wo, bo):
    """Host-side packed constants. Returns (wr_pack, mats, biasp)."""
    f32 = np.float32
    Wr = np.asarray(Wr, f32)
    br = np.asarray(br, f32)
    We = np.asarray(We, f32)
    be = np.asarray(be, f32)
    Wq = np.asarray(Wq, f32)
    bq = np.asarray(bq, f32)
    Wk = np.asarray(Wk, f32)
    bk = np.asarray(bk, f32)
    Wv = np.asarray(Wv, f32)
    bv = np.asarray(bv, f32)
    Wo = np.asarray(Wo, f32)
    bo = np.asarray(bo, f32)

    # router weight: [TDP, 2, GE] bf16 (hi, lo planes), K-padded
    wr_pad = np.zeros((TDP, GE), f32)
    wr_pad[:TD] = Wr
    whi, wlo = bf16pair(wr_pad)
    wr_pack = np.zeros((TDP, 2, GE), NPBF)
    wr_pack[:, 0, :] = whi
    wr_pack[:, 1, :] = wlo

    mats = np.zeros((NMATS, GE, GE), f32)
    biasp = np.zeros((GE, NBIAS), f32)

    wet = np.transpose(We, (2, 0, 1)).reshape(GE, GE)
    be_vec = be.reshape(GE)
    biasp[:, B_BO] = bo.reshape(GE)
    biasp[:, B_BR] = br

    aq = np.zeros((GE, GE), f32)
    ak = np.zeros((DH, GE, GE), f32)
    av = np.zeros((DH, GE, GE), f32)
    bqp = np.zeros(GE, f32)
    bkp = np.zeros((DH, GE), f32)
    bvp = np.zeros((DH, GE), f32)
    ms = np.zeros((DH, GE, GE), f32)
    d_ = np.arange(DH)
    for g in range(G):
        for h in range(H):
            for d in range(DH):
                p = d * 32 + g * 4 + h
                aq[g * 16:(g + 1) * 16, p] = Wq[g, h * 4 + d, :]
                bqp[p] = bq[g, h * 4 + d]
            for e in range(DH):
                for p in d_ * 32 + g * 4 + h:
                    ak[e, g * 16:(g + 1) * 16, p] = Wk[g, h * 4 + e, :]
                    av[e, g * 16:(g + 1) * 16, p] = Wv[g, h * 4 + e, :]
                    bkp[e, p] = bk[g, h * 4 + e]
                    bvp[e, p] = bv[g, h * 4 + e]
    # expert-layer fold: y = A^T(WET^T h + be) + b -> (WET@A)^T h + (A^T be + b)
    mats[M_AQ] = wet @ aq
    bqp += aq.T @ be_vec
    for e in range(DH):
        mats[M_AK0 + e] = wet @ ak[e]
        mats[M_AV0 + e] = wet @ av[e]
        bkp[e] += ak[e].T @ be_vec
        bvp[e] += av[e].T @ be_vec

    for e in range(DH):
        for d in range(DH):
            for g in range(G):
                for h in range(H):
                    ms[e, d * 32 + g * 4 + h, e * 32 + d * 8 + g] = 1.0
                    mats[M_MDEN, e * 32 + d * 8 + g, d * 32 + g * 4 + h] = 1.0
                    mats[M_MER0 + e, e * 32 + d * 8 + g, d * 32 + g * 4 + h] = 1.0
        mats[M_MS0 + e] = ms[e]
    # q/k bias folds: scores = Sum_e MS_e^T(q*k_e) + MSCH^T h + const
    mats[M_MSCH] = mats[M_AQ] @ sum(bkp[e][:, None] * ms[e] for e in range(DH)) \
        + sum(mats[M_AK0 + e] @ (bqp[:, None] * ms[e]) for e in range(DH))
    sc_const = sum(ms[e].T @ (bqp * bkp[e]) for e in range(DH))
    biasp[:, B_SC] = 0.5 * sc_const
    # v bias fold: att_num = Sum_e er_e*v_e + MBV^T es
    mats[M_MBV] = sum(mats[M_MER0 + e] * bvp[e][None, :] for e in range(DH))
    for g in range(G):
        for f in range(E):
            for h in range(H):
                for d in range(DH):
                    mats[M_AO, d * 32 + g * 4 + h, g * 16 + f] = Wo[g, f, h * 4 + d]

    mats_packed = np.ascontiguousarray(
        np.transpose(mats, (1, 0, 2)).reshape(GE, NMATS * GE)).astype(np.float16)
    return wr_pack, mats_packed, biasp


def pack_x_shard(xs):
    """[NSH, TD] fp32 -> flat bf16 (block, ktile, part, plane, col) stream."""
    xt = np.zeros(TD * NSH * 2, NPBF)
    for c, w in enumerate(WIDTHS):
        hi, lo = bf16pair(xs[OFFS[c]:OFFS[c] + w].T)   # [TD, w] each
        blk = np.empty((TD, 2, w), NPBF)
        blk[:, 0, :] = hi
        blk[:, 1, :] = lo
        o = 2 * TD * OFFS[c]
        xt[o:o + 2 * TD * w] = blk.ravel()
    return xt


def build_kernel():
    """Trace the Bass/Tile kernel; returns the compiled Bacc."""
    nc = bacc.Bacc("TRN2", target_bir_lowering=False, debug=False,
                   num_devices=NCORES)

    xt_d = nc.dram_tensor("xt", [TD * NSH * 2], BF16, kind="ExternalInput").ap()
    wr_d = nc.dram_tensor("wr", [TDP, 2 * GE], BF16, kind="ExternalInput").ap()
    mats_d = nc.dram_tensor("mats", [GE, NMATS * GE], F16, kind="ExternalInput").ap()
    bias_d = nc.dram_tensor("bias", [GE, NBIAS], F32, kind="ExternalInput").ap()
    out_d = nc.dram_tensor("out", [NSH], F32, kind="ExternalOutput").ap()

    RMAX = bass_isa.ReduceOp.max
    RADD = bass_isa.ReduceOp.add

    with tile.TileContext(nc) as tc:
        with (
            nc.allow_low_precision(reason="bf16 value path is validated to "
                                   "2.7e-3 rel err; gating stays fp32"),
            tc.tile_pool(name="consts", bufs=1) as consts,
            tc.tile_pool(name="xts", bufs=5) as xts,
            tc.tile_pool(name="work", bufs=2) as work,
            tc.tile_pool(name="ps", bufs=6, space="PSUM") as ptp,
            tc.tile_pool(name="hp", bufs=2, space="PSUM") as htp,
        ):
            # ---- constants ----
            # wr streams in per-K-group pieces interleaved with block 0's x
            # subs (a single up-front 4.9MB wr DMA would delay the first
            # router matmul by ~14us); mats/bias follow block 0's first sub.
            wr_sb = consts.tile([128, KT + 1, 2 * GE], BF16, tag="wr")
            wr_r = wr_d.rearrange("(c p) m -> p c m", p=128)
            mats_sb = consts.tile([GE, NMATS * GE], F16, tag="mats")
            bias_sb = consts.tile([GE, NBIAS], F32, tag="bias")

            def mat(i):
                return mats_sb[:, i * GE:(i + 1) * GE]

            def whi(k, p=128):
                return wr_sb[0:p, k, 0:GE]

            def wlo(k, p=128):
                return wr_sb[0:p, k, GE:2 * GE]

            def bcol(i):
                return bias_sb[:, i:i + 1]

            def wtile(tag, c, dt=F32, p=GE):
                return work.tile([p, WMAX], dt, tag=tag, name=f"{tag}{c}")

            def post_phases(c, hps):
                """Post-processing split into 4 phases so the PE stream can
                interleave the next block's accumulation matmuls between the
                chain's cross-engine stalls."""
                w = WIDTHS[c]
                st = {}

                def A():
                    h_sb = wtile("h", c)[:, :w]
                    nc.scalar.activation(h_sb, hps[:, :w],
                                         mybir.ActivationFunctionType.Identity,
                                         bias=bcol(B_BR), scale=1.0)
                    eh = wtile("eh", c)[:, :w]
                    nc.scalar.activation(eh, hps[:, :w],
                                         mybir.ActivationFunctionType.Exp,
                                         bias=bcol(B_BR), scale=1.0)
                    hb = wtile("hb", c, F16)[:, :w]
                    nc.scalar.activation(hb, hps[:, :w],
                                         mybir.ActivationFunctionType.Identity,
                                         bias=bcol(B_BR), scale=1.0)
                    # exact fp32 top-2 gating, entirely on the Pool engine
                    mx1b = wtile("mx1b", c)[:, :w]
                    nc.gpsimd.partition_all_reduce(mx1b, h_sb, channels=GE,
                                                   reduce_op=RMAX)
                    eqm = wtile("eqm", c)[:, :w]
                    nc.vector.tensor_tensor(eqm, h_sb, mx1b,
                                            op=mybir.AluOpType.is_ge)
                    hm = wtile("hm", c)[:, :w]
                    nc.vector.scalar_tensor_tensor(
                        hm, in0=eqm, scalar=-1e30, in1=h_sb,
                        op0=mybir.AluOpType.mult, op1=mybir.AluOpType.add)
                    mx2b = wtile("mx2b", c)[:, :w]
                    nc.gpsimd.partition_all_reduce(mx2b, hm, channels=GE,
                                                   reduce_op=RMAX)
                    m1 = wtile("m1", c)[:, :w]
                    nc.vector.tensor_tensor(m1, h_sb, mx2b,
                                            op=mybir.AluOpType.is_ge)
                    nc.vector.tensor_mul(m1, m1, eh)
                    sdenb = wtile("sdenb", c)[:, :w]
                    nc.gpsimd.partition_all_reduce(sdenb, m1, channels=GE,
                                                   reduce_op=RADD)
                    rden = wtile("rden", c, p=1)[:, :w]
                    nc.vector.reciprocal(rden, sdenb[0:1, :w])
                    st["rden"] = rden
                    # all h-fed matmuls up front: PE never waits here
                    q_ps = ptp.tile([GE, WMAX], F32, tag="pt", name=f"q{c}")
                    nc.tensor.matmul(q_ps[:, :w], lhsT=mat(M_AQ), rhs=hb,
                                     start=True, stop=True)
                    k_ps = [ptp.tile([GE, WMAX], F32, tag="pt", name=f"k{c}_{e}")
                            for e in range(DH)]
                    for e in range(DH):
                        nc.tensor.matmul(k_ps[e][:, :w], lhsT=mat(M_AK0 + e),
                                         rhs=hb, start=True, stop=True)
                    # HW: DVE reads at most one PSUM operand, so q drains
                    # through the (mostly idle) ACT engine first
                    qb = wtile("qb", c, F16)[:, :w]
                    nc.scalar.activation(qb, q_ps[:, :w],
                                         mybir.ActivationFunctionType.Identity)
                    pes = []
                    for e in range(DH):
                        pe_sb = wtile(f"pe{e % 2}", c, F16)[:, :w]
                        nc.vector.tensor_mul(pe_sb, qb, k_ps[e][:, :w])
                        pes.append(pe_sb)
                    st.update(hb=hb, m1=m1, sdenb=sdenb, pes=pes)

                def B():
                    sc_ps = ptp.tile([GE, WMAX], F32, tag="pt", name=f"sc{c}")
                    nc.tensor.matmul(sc_ps[:, :w], lhsT=mat(M_MSCH),
                                     rhs=st["hb"], start=True, stop=False)
                    for e in range(DH):
                        nc.tensor.matmul(sc_ps[:, :w], lhsT=mat(M_MS0 + e),
                                         rhs=st["pes"][e],
                                         start=False, stop=(e == DH - 1))
                    es_sb = wtile("es", c, F16)[:, :w]
                    nc.scalar.activation(es_sb, sc_ps[:, :w],
                                         mybir.ActivationFunctionType.Exp,
                                         bias=bcol(B_SC), scale=0.5)
                    st["es"] = es_sb

                def C():
                    es_sb = st["es"]
                    den_ps = ptp.tile([GE, WMAX], F32, tag="pt", name=f"den{c}")
                    nc.tensor.matmul(den_ps[:, :w], lhsT=mat(M_MDEN),
                                     rhs=es_sb, start=True, stop=True)
                    drec = wtile("drec", c, F16)[:, :w]
                    nc.vector.reciprocal(drec, den_ps[:, :w])
                    prs = []
                    for e in range(DH):
                        er_ps = ptp.tile([GE, WMAX], F32, tag="pt",
                                         name=f"er{c}_{e}")
                        nc.tensor.matmul(er_ps[:, :w], lhsT=mat(M_MER0 + e),
                                         rhs=es_sb, start=True, stop=True)
                        v_ps = ptp.tile([GE, WMAX], F32, tag="pt",
                                        name=f"v{c}_{e}")
                        nc.tensor.matmul(v_ps[:, :w], lhsT=mat(M_AV0 + e),
                                         rhs=st["hb"], start=True, stop=True)
                        vb = wtile(f"vb{e % 2}", c, F16)[:, :w]
                        nc.scalar.activation(vb, v_ps[:, :w],
                                             mybir.ActivationFunctionType.Identity)
                        pr = wtile(f"pr{e % 2}", c, F16)[:, :w]
                        nc.vector.tensor_mul(pr, er_ps[:, :w], vb)
                        prs.append(pr)
                    bv_ps = ptp.tile([GE, WMAX], F32, tag="pt", name=f"bv{c}")
                    nc.tensor.matmul(bv_ps[:, :w], lhsT=mat(M_MBV), rhs=es_sb,
                                     start=True, stop=True)
                    t01 = wtile("t01", c, F16)[:, :w]
                    nc.vector.tensor_add(t01, prs[0], prs[1])
                    t23 = wtile("t23", c, F16)[:, :w]
                    nc.vector.tensor_add(t23, prs[2], prs[3])
                    att = wtile("att", c, F16)[:, :w]
                    nc.vector.tensor_add(att, t01, t23)
                    nc.vector.tensor_add(att, att, bv_ps[:, :w])
                    nc.vector.tensor_mul(att, att, drec)
                    st["att"] = att

                def D():
                    ao_ps = ptp.tile([GE, WMAX], F32, tag="pt", name=f"ao{c}")
                    nc.tensor.matmul(ao_ps[:, :w], lhsT=mat(M_AO),
                                     rhs=st["att"], start=True, stop=True)
                    aout = wtile("aout", c)[:, :w]
                    nc.vector.tensor_scalar_add(aout, ao_ps[:, :w], bcol(B_BO))
                    num = wtile("num", c)[:, :w]
                    nc.vector.tensor_mul(num, st["m1"], aout)
                    snumb = wtile("snumb", c)[:, :w]
                    nc.gpsimd.partition_all_reduce(snumb, num, channels=GE,
                                                   reduce_op=RADD)
                    pred = wtile("pred", c, p=1)[:, :w]
                    nc.vector.tensor_mul(pred, snumb[0:1, :w], st["rden"])
                    # ACT's DGE queue: an SP-issued DMA would park the SP
                    # sequencer on pred's semaphore and stall the x stream
                    nc.scalar.dma_start(out=out_d[OFFS[c]:OFFS[c] + w],
                                        in_=pred)

                return [A, B, C, D]

            # ---- block pipeline: stream block c+1 while post(c) runs.
            # post(c-1)'s phases are interleaved between block c's sub-DMA
            # accumulation groups so the chain's cross-engine stalls never
            # block the accumulation matmuls in the in-order PE stream.
            phases = []
            for c, w in enumerate(WIDTHS):
                hps = htp.tile([GE, WMAX], F32, tag="hps", name=f"hps{c}")
                base = 2 * TD * OFFS[c]

                def accum(hi_ap, lo_ap, k, p=128):
                    """3-pass bf16 split-precision accumulation for K-tile k."""
                    nc.tensor.matmul(hps[:, :w], lhsT=whi(k, p), rhs=hi_ap,
                                     start=(k == 0), stop=False)
                    nc.tensor.matmul(hps[:, :w], lhsT=wlo(k, p), rhs=hi_ap,
                                     start=False, stop=False)
                    nc.tensor.matmul(hps[:, :w], lhsT=whi(k, p), rhs=lo_ap,
                                     start=False, stop=(k == KT))

                k0 = 0
                for si, nk in enumerate(ksubs_for(w, last=(c == len(WIDTHS) - 1))):
                    if c == 0:
                        nwr = (KT + 1 - k0) if k0 + nk >= KT else nk
                        nc.sync.dma_start(out=wr_sb[:, k0:k0 + nwr, :],
                                          in_=wr_r[:, k0:k0 + nwr, :])
                    xs = xts.tile([128, 5120], BF16, tag="xt")
                    xv = xs[:, :nk * 2 * w].rearrange("p (k q m) -> p k q m",
                                                      k=nk, q=2)
                    src = xt_d[base + k0 * 256 * w:
                               base + (k0 + nk) * 256 * w]
                    nc.sync.dma_start(
                        out=xv, in_=src.rearrange("(k p q m) -> p k q m",
                                                  p=128, q=2, m=w))
                    for t in range(nk):
                        accum(xv[:, t, 0, :], xv[:, t, 1, :], k0 + t)
                    k0 += nk
                    if c == 0 and si == 0:
                        nc.sync.dma_start(out=mats_sb, in_=mats_d)
                        nc.sync.dma_start(out=bias_sb, in_=bias_d)
                    if si >= 1 and phases:
                        phases.pop(0)()
                # 8-row contraction remainder (rows 74*128 .. TD)
                xs8 = xts.tile([128, 5120], BF16, tag="xt", name=f"x8_{c}")
                xv8 = xs8[0:KREM, :2 * w].rearrange("p (q m) -> p q m", q=2)
                src8 = xt_d[base + KT * 256 * w:base + KT * 256 * w + KREM * 2 * w]
                nc.sync.dma_start(
                    out=xv8, in_=src8.rearrange("(p q m) -> p q m",
                                                p=KREM, q=2, m=w))
                accum(xv8[:, 0, :], xv8[:, 1, :], KT, p=KREM)
                while phases:
                    phases.pop(0)()
                phases = post_phases(c, hps)
            for ph in phases:
                ph()

    nc.compile()
    return nc


_NC_CACHE = None
LAST_RESULTS = None


def kernel(x, Wr, br, We, be, Wq, bq, Wk, bk, Wv, bv, Wo, bo):
    global _NC_CACHE, LAST_RESULTS
    f32 = np.float32
    x = np.asarray(x, f32)

    wr_pack, mats_packed, biasp = build_consts(
        Wr, br, We, be, Wq, bq, Wk, bk, Wv, bv, Wo, bo)

    if _NC_CACHE is None:
        _NC_CACHE = build_kernel()
    nc = _NC_CACHE

    in_maps = []
    for core in range(NCORES):
        xs = x[core * NSH:(core + 1) * NSH].reshape(NSH, TD)
        in_maps.append({"xt": pack_x_shard(xs),
                        "wr": wr_pack.reshape(TDP, 2 * GE),
                        "mats": mats_packed, "bias": biasp})

    res = run_bass_kernel_spmd(nc, in_maps, list(range(NCORES)))
    LAST_RESULTS = res
    out = np.concatenate([res.results[core]["out"].reshape(NSH)
                          for core in range(NCORES)])
    return out.astype(f32)


# revision 32
# speedup vs baseline: 1.0021x; 1.0021x over previous
"""Trainium2 Bass kernel for nn_MIGAModel (moe_routing).

Strategy (pure data parallel over the stock axis N, 8 cores):
 - The x stream (94.8MB/core, ~264us at 360GB/s) is the roofline;
   everything else hides under it.  x ships as bf16 (hi, lo) plane
   pairs (same bytes as fp32, exact to ~2^-18), pre-transposed and
   column-blocked on host so each block is contiguous in DRAM.
 - Router h = Whi.xhi + Wlo.xhi + Whi.xlo: three bf16 matmuls per
   K-tile at 1 PE cycle/row (vs 4 for fp32) accumulate in fp32 PSUM.
   h error ~3e-6 -> zero top-2 flips (gating ranking is the
   precision-critical part: plain bf16/fp16 routing fails the 2e-2
   gate; plain fp32 matmuls would make PE the bottleneck; fp32r needs
   pre-rounded inputs on real HW, i.e. is not fp32-accurate).
 - Column-block pipeline: while block c+1's x streams, block c is
   post-processed; trailing blocks are narrower so the only exposed
   post-chain (the last block's) is short.
 - Host algebra folds the expert layer into Q/K/V (eo never exists),
   all q/k biases into one score matmul on h (MSCH) + a constant in
   the exp-activation bias, and the v biases into one matmul on the
   score numerators (MBV).  The value path runs in fp16 (error
   ~1e-3, gating unaffected; HW rel err ~1.25e-2 is dominated by a
   couple of top-2 near-tie flips from the router's ~3e-6 h-noise).
 - Gating and the final weighted combine use Pool-engine
   partition_all_reduce (max/add across the 128 expert partitions):
   no transposes or broadcast matmuls; the reduces overlap the DVE
   compare/select and value-path ops (real HW restricts Pool to the
   reduce family, DVE to one PSUM operand per op, and has no DVE
   divide -- hence the ACT-engine drains and recip+mul).
"""
import sys
import numpy as np
import ml_dtypes

for _p in ("/opt/trn_rl_repo",):
    if _p not in sys.path:
        sys.path.insert(0, _p)

import concourse.bass as bass
import concourse.tile as tile
from concourse import bacc, mybir, bass_isa
from concourse.bass_utils import run_bass_kernel_spmd

F32 = mybir.dt.float32
BF16 = mybir.dt.bfloat16
F16 = mybir.dt.float16
NPBF = ml_dtypes.bfloat16

N, T, D = 20000, 60, 158
TD = T * D                      # 9480
G, E, H, DH, GE = 8, 16, 4, 4, 128
NCORES = 8
NSH = N // NCORES               # 2500 rows per core
KT = TD // 128                  # 74 full K-tiles
KREM = TD - KT * 128            # 8 remainder contraction rows
TDP = (KT + 1) * 128            # wr ships padded to 75 tiles

# column blocks: sum = NSH; >=256 keeps DMA lines >=1KB; narrow
# trailing blocks shorten the exposed final post-chain
WIDTHS = [512, 512, 512, 452, 256, 128, 128]
assert sum(WIDTHS) == NSH and all(128 <= w <= 512 for w in WIDTHS)
WMAX = max(WIDTHS)
OFFS = [sum(WIDTHS[:i]) for i in range(len(WIDTHS))]


def ksubs_for(w, last=False):
    """K-tiles per sub-DMA over the 74 full tiles: ~10KB SBUF tiles.
    The final block tapers so almost no accumulation trails the stream."""
    nk = max(1, 2560 // w)
    ks = [nk] * (KT // nk)
    if KT % nk:
        ks.append(KT % nk)
    if last and ks[-1] > 4:
        ks[-1:] = [ks[-1] - 4, 4]
    return ks


# packed matrix indices (each a [128,128] block in the "mats" input)
M_AQ = 0
M_AK0, M_AV0 = 1, 5             # 4 each
M_MS0 = 9                       # 4
M_MSCH = 13
M_MDEN = 14
M_MBV = 15
M_MER0 = 16                     # 4
M_AO = 20
NMATS = 21

# bias pack columns
B_BO, B_BR, B_SC = 0, 1, 2
NBIAS = 4


def bf16pair(a):
    """Split fp32 array into (hi, lo) bf16 planes, hi+lo ~= a to 2^-18."""
    a = np.asarray(a, np.float32)
    hi = a.astype(NPBF)
    lo = (a - hi.astype(np.float32)).astype(NPBF)
    return hi, lo


def build_consts(Wr, br, We, be, Wq, bq, Wk, bk, Wv, bv, Wo, bo):
    """Host-side packed constants. Returns (wr_pack, mats, biasp)."""
    f32 = np.float32
    Wr = np.asarray(Wr, f32)
    br = np.asarray(br, f32)
    We = np.asarray(We, f32)
    be = np.asarray(be, f32)
    Wq = np.asarray(Wq, f32)
    bq = np.asarray(bq, f32)
    Wk = np.asarray(Wk, f32)
    bk = np.asarray(bk, f32)
    Wv = np.asarray(Wv, f32)
    bv = np.asarray(bv, f32)
    Wo = np.asarray(Wo, f32)
    bo = np.asarray(bo, f32)

    # router weight: [TDP, 2, GE] bf16 (hi, lo planes), K-padded
    wr_pad = np.zeros((TDP, GE), f32)
    wr_pad[:TD] = Wr
    whi, wlo = bf16pair(wr_pad)
    wr_pack = np.zeros((TDP, 2, GE), NPBF)
    wr_pack[:, 0, :] = whi
    wr_pack[:, 1, :] = wlo

    mats = np.zeros((NMATS, GE, GE), f32)
    biasp = np.zeros((GE, NBIAS), f32)

    wet = np.transpose(We, (2, 0, 1)).reshape(GE, GE)
    be_vec = be.reshape(GE)
    biasp[:, B_BO] = bo.reshape(GE)
    biasp[:, B_BR] = br

    aq = np.zeros((GE, GE), f32)
    ak = np.zeros((DH, GE, GE), f32)
    av = np.zeros((DH, GE, GE), f32)
    bqp = np.zeros(GE, f32)
    bkp = np.zeros((DH, GE), f32)
    bvp = np.zeros((DH, GE), f32)
    ms = np.zeros((DH, GE, GE), f32)
    d_ = np.arange(DH)
    for g in range(G):
        for h in range(H):
            for d in range(DH):
                p = d * 32 + g * 4 + h
                aq[g * 16:(g + 1) * 16, p] = Wq[g, h * 4 + d, :]
                bqp[p] = bq[g, h * 4 + d]
            for e in range(DH):
                for p in d_ * 32 + g * 4 + h:
                    ak[e, g * 16:(g + 1) * 16, p] = Wk[g, h * 4 + e, :]
                    av[e, g * 16:(g + 1) * 16, p] = Wv[g, h * 4 + e, :]
                    bkp[e, p] = bk[g, h * 4 + e]
                    bvp[e, p] = bv[g, h * 4 + e]
    # expert-layer fold: y = A^T(WET^T h + be) + b -> (WET@A)^T h + (A^T be + b)
    mats[M_AQ] = wet @ aq
    bqp += aq.T @ be_vec
    for e in range(DH):
        mats[M_AK0 + e] = wet @ ak[e]
        mats[M_AV0 + e] = wet @ av[e]
        bkp[e] += ak[e].T @ be_vec
        bvp[e] += av[e].T @ be_vec

    for e in range(DH):
        for d in range(DH):
            for g in range(G):
                for h in range(H):
                    ms[e, d * 32 + g * 4 + h, e * 32 + d * 8 + g] = 1.0
                    mats[M_MDEN, e * 32 + d * 8 + g, d * 32 + g * 4 + h] = 1.0
                    mats[M_MER0 + e, e * 32 + d * 8 + g, d * 32 + g * 4 + h] = 1.0
        mats[M_MS0 + e] = ms[e]
    # q/k bias folds: scores = Sum_e MS_e^T(q*k_e) + MSCH^T h + const
    mats[M_MSCH] = mats[M_AQ] @ sum(bkp[e][:, None] * ms[e] for e in range(DH)) \
        + sum(mats[M_AK0 + e] @ (bqp[:, None] * ms[e]) for e in range(DH))
    sc_const = sum(ms[e].T @ (bqp * bkp[e]) for e in range(DH))
    biasp[:, B_SC] = 0.5 * sc_const
    # v bias fold: att_num = Sum_e er_e*v_e + MBV^T es
    mats[M_MBV] = sum(mats[M_MER0 + e] * bvp[e][None, :] for e in range(DH))
    for g in range(G):
        for f in range(E):
            for h in range(H):
                for d in range(DH):
                    mats[M_AO, d * 32 + g * 4 + h, g * 16 + f] = Wo[g, f, h * 4 + d]

    mats_packed = np.ascontiguousarray(
        np.transpose(mats, (1, 0, 2)).reshape(GE, NMATS * GE)).astype(np.float16)
    return wr_pack, mats_packed, biasp


def pack_x_shard(xs):
    """[NSH, TD] fp32 -> flat bf16 (block, ktile, part, plane, col) stream."""
    xt = np.zeros(TD * NSH * 2, NPBF)
    for c, w in enumerate(WIDTHS):
        hi, lo = bf16pair(xs[OFFS[c]:OFFS[c] + w].T)   # [TD, w] each
        blk = np.empty((TD, 2, w), NPBF)
        blk[:, 0, :] = hi
        blk[:, 1, :] = lo
        o = 2 * TD * OFFS[c]
        xt[o:o + 2 * TD * w] = blk.ravel()
    return xt


def build_kernel():
    """Trace the Bass/Tile kernel; returns the compiled Bacc."""
    nc = bacc.Bacc("TRN2", target_bir_lowering=False, debug=False,
                   num_devices=NCORES)

    xt_d = nc.dram_tensor("xt", [TD * NSH * 2], BF16, kind="ExternalInput").ap()
    wr_d = nc.dram_tensor("wr", [TDP, 2 * GE], BF16, kind="ExternalInput").ap()
    mats_d = nc.dram_tensor("mats", [GE, NMATS * GE], F16, kind="ExternalInput").ap()
    bias_d = nc.dram_tensor("bias", [GE, NBIAS], F32, kind="ExternalInput").ap()
    out_d = nc.dram_tensor("out", [NSH], F32, kind="ExternalOutput").ap()

    RMAX = bass_isa.ReduceOp.max
    RADD = bass_isa.ReduceOp.add

    with tile.TileContext(nc) as tc:
        with (
            nc.allow_low_precision(reason="bf16 value path is validated to "
                                   "2.7e-3 rel err; gating stays fp32"),
            tc.tile_pool(name="consts", bufs=1) as consts,
            tc.tile_pool(name="xts", bufs=6) as xts,
            tc.tile_pool(name="work", bufs=2) as work,
            tc.tile_pool(name="ps", bufs=6, space="PSUM") as ptp,
            tc.tile_pool(name="hp", bufs=2, space="PSUM") as htp,
        ):
            # ---- constants ----
            # wr streams in per-K-group pieces interleaved with block 0's x
            # subs (a single up-front 4.9MB wr DMA would delay the first
            # router matmul by ~14us); mats/bias follow block 0's first sub.
            wr_sb = consts.tile([128, KT + 1, 2 * GE], BF16, tag="wr")
            wr_r = wr_d.rearrange("(c p) m -> p c m", p=128)
            mats_sb = consts.tile([GE, NMATS * GE], F16, tag="mats")
            bias_sb = consts.tile([GE, NBIAS], F32, tag="bias")

            def mat(i):
                return mats_sb[:, i * GE:(i + 1) * GE]

            def whi(k, p=128):
                return wr_sb[0:p, k, 0:GE]

            def wlo(k, p=128):
                return wr_sb[0:p, k, GE:2 * GE]

            def bcol(i):
                return bias_sb[:, i:i + 1]

            def wtile(tag, c, dt=F32, p=GE):
                return work.tile([p, WMAX], dt, tag=tag, name=f"{tag}{c}")

            def post_phases(c, hps, tail=False):
                """Post-processing split into 4 phases so the PE stream can
                interleave the next block's accumulation matmuls between the
                chain's cross-engine stalls.  For the exposed final block
                (tail=True) the value path is emitted before the gating ops,
                since there the chain latency itself is the kernel tail."""
                w = WIDTHS[c]
                st = {}

                def gating(h_sb, eh):
                    # exact fp32 top-2 gating: reduces on Pool, the rest DVE
                    mx1b = wtile("mx1b", c)[:, :w]
                    nc.gpsimd.partition_all_reduce(mx1b, h_sb, channels=GE,
                                                   reduce_op=RMAX)
                    eqm = wtile("eqm", c)[:, :w]
                    nc.vector.tensor_tensor(eqm, h_sb, mx1b,
                                            op=mybir.AluOpType.is_ge)
                    hm = wtile("hm", c)[:, :w]
                    nc.vector.scalar_tensor_tensor(
                        hm, in0=eqm, scalar=-1e30, in1=h_sb,
                        op0=mybir.AluOpType.mult, op1=mybir.AluOpType.add)
                    mx2b = wtile("mx2b", c)[:, :w]
                    nc.gpsimd.partition_all_reduce(mx2b, hm, channels=GE,
                                                   reduce_op=RMAX)
                    m1 = wtile("m1", c)[:, :w]
                    nc.vector.tensor_tensor(m1, h_sb, mx2b,
                                            op=mybir.AluOpType.is_ge)
                    nc.vector.tensor_mul(m1, m1, eh)
                    sdenb = wtile("sdenb", c)[:, :w]
                    nc.gpsimd.partition_all_reduce(sdenb, m1, channels=GE,
                                                   reduce_op=RADD)
                    rden = wtile("rden", c, p=1)[:, :w]
                    nc.vector.reciprocal(rden, sdenb[0:1, :w])
                    st.update(m1=m1, sdenb=sdenb, rden=rden)

                def heads(hb):
                    # all h-fed matmuls up front: PE never waits here
                    q_ps = ptp.tile([GE, WMAX], F32, tag="pt", name=f"q{c}")
                    nc.tensor.matmul(q_ps[:, :w], lhsT=mat(M_AQ), rhs=hb,
                                     start=True, stop=True)
                    k_ps = [ptp.tile([GE, WMAX], F32, tag="pt", name=f"k{c}_{e}")
                            for e in range(DH)]
                    for e in range(DH):
                        nc.tensor.matmul(k_ps[e][:, :w], lhsT=mat(M_AK0 + e),
                                         rhs=hb, start=True, stop=True)
                    # HW: DVE reads at most one PSUM operand, so q drains
                    # through the (mostly idle) ACT engine first
                    qb = wtile("qb", c, F16)[:, :w]
                    nc.scalar.activation(qb, q_ps[:, :w],
                                         mybir.ActivationFunctionType.Identity)
                    pes = []
                    for e in range(DH):
                        pe_sb = wtile(f"pe{e % 2}", c, F16)[:, :w]
                        nc.vector.tensor_mul(pe_sb, qb, k_ps[e][:, :w])
                        pes.append(pe_sb)
                    if tail:
                        # pull the v matmuls + drains off the exposed C chain
                        vbs = []
                        for e in range(DH):
                            v_ps = ptp.tile([GE, WMAX], F32, tag="pt",
                                            name=f"v{c}_{e}")
                            nc.tensor.matmul(v_ps[:, :w], lhsT=mat(M_AV0 + e),
                                             rhs=hb, start=True, stop=True)
                            vb = wtile(f"vb{e % 2}", c, F16)[:, :w]
                            nc.scalar.activation(
                                vb, v_ps[:, :w],
                                mybir.ActivationFunctionType.Identity)
                            vbs.append(vb)
                        st["vbs"] = vbs
                    st.update(hb=hb, pes=pes)

                def A():
                    h_sb = wtile("h", c)[:, :w]
                    nc.scalar.activation(h_sb, hps[:, :w],
                                         mybir.ActivationFunctionType.Identity,
                                         bias=bcol(B_BR), scale=1.0)
                    if tail:
                        hb = wtile("hb", c, F16)[:, :w]
                        nc.scalar.activation(hb, hps[:, :w],
                                             mybir.ActivationFunctionType.Identity,
                                             bias=bcol(B_BR), scale=1.0)
                        eh = wtile("eh", c)[:, :w]
                        nc.scalar.activation(eh, hps[:, :w],
                                             mybir.ActivationFunctionType.Exp,
                                             bias=bcol(B_BR), scale=1.0)
                        heads(hb)
                        gating(h_sb, eh)
                    else:
                        eh = wtile("eh", c)[:, :w]
                        nc.scalar.activation(eh, hps[:, :w],
                                             mybir.ActivationFunctionType.Exp,
                                             bias=bcol(B_BR), scale=1.0)
                        hb = wtile("hb", c, F16)[:, :w]
                        nc.scalar.activation(hb, hps[:, :w],
                                             mybir.ActivationFunctionType.Identity,
                                             bias=bcol(B_BR), scale=1.0)
                        gating(h_sb, eh)
                        heads(hb)

                def B():
                    sc_ps = ptp.tile([GE, WMAX], F32, tag="pt", name=f"sc{c}")
                    nc.tensor.matmul(sc_ps[:, :w], lhsT=mat(M_MSCH),
                                     rhs=st["hb"], start=True, stop=False)
                    for e in range(DH):
                        nc.tensor.matmul(sc_ps[:, :w], lhsT=mat(M_MS0 + e),
                                         rhs=st["pes"][e],
                                         start=False, stop=(e == DH - 1))
                    es_sb = wtile("es", c, F16)[:, :w]
                    nc.scalar.activation(es_sb, sc_ps[:, :w],
                                         mybir.ActivationFunctionType.Exp,
                                         bias=bcol(B_SC), scale=0.5)
                    st["es"] = es_sb

                def C():
                    es_sb = st["es"]
                    den_ps = ptp.tile([GE, WMAX], F32, tag="pt", name=f"den{c}")
                    nc.tensor.matmul(den_ps[:, :w], lhsT=mat(M_MDEN),
                                     rhs=es_sb, start=True, stop=True)
                    drec = wtile("drec", c, F16)[:, :w]
                    nc.vector.reciprocal(drec, den_ps[:, :w])
                    prs = []
                    for e in range(DH):
                        er_ps = ptp.tile([GE, WMAX], F32, tag="pt",
                                         name=f"er{c}_{e}")
                        nc.tensor.matmul(er_ps[:, :w], lhsT=mat(M_MER0 + e),
                                         rhs=es_sb, start=True, stop=True)
                        if tail:
                            vb = st["vbs"][e]
                        else:
                            v_ps = ptp.tile([GE, WMAX], F32, tag="pt",
                                            name=f"v{c}_{e}")
                            nc.tensor.matmul(v_ps[:, :w], lhsT=mat(M_AV0 + e),
                                             rhs=st["hb"], start=True, stop=True)
                            vb = wtile(f"vb{e % 2}", c, F16)[:, :w]
                            nc.scalar.activation(
                                vb, v_ps[:, :w],
                                mybir.ActivationFunctionType.Identity)
                        pr = wtile(f"pr{e % 2}", c, F16)[:, :w]
                        nc.vector.tensor_mul(pr, er_ps[:, :w], vb)
                        prs.append(pr)
                    bv_ps = ptp.tile([GE, WMAX], F32, tag="pt", name=f"bv{c}")
                    nc.tensor.matmul(bv_ps[:, :w], lhsT=mat(M_MBV), rhs=es_sb,
                                     start=True, stop=True)
                    t01 = wtile("t01", c, F16)[:, :w]
                    nc.vector.tensor_add(t01, prs[0], prs[1])
                    t23 = wtile("t23", c, F16)[:, :w]
                    nc.vector.tensor_add(t23, prs[2], prs[3])
                    att = wtile("att", c, F16)[:, :w]
                    nc.vector.tensor_add(att, t01, t23)
                    nc.vector.tensor_add(att, att, bv_ps[:, :w])
                    nc.vector.tensor_mul(att, att, drec)
                    st["att"] = att

                def D():
                    ao_ps = ptp.tile([GE, WMAX], F32, tag="pt", name=f"ao{c}")
                    nc.tensor.matmul(ao_ps[:, :w], lhsT=mat(M_AO),
                                     rhs=st["att"], start=True, stop=True)
                    aout = wtile("aout", c)[:, :w]
                    nc.vector.tensor_scalar_add(aout, ao_ps[:, :w], bcol(B_BO))
                    num = wtile("num", c)[:, :w]
                    nc.vector.tensor_mul(num, st["m1"], aout)
                    snumb = wtile("snumb", c)[:, :w]
                    nc.gpsimd.partition_all_reduce(snumb, num, channels=GE,
                                                   reduce_op=RADD)
                    pred = wtile("pred", c, p=1)[:, :w]
                    nc.vector.tensor_mul(pred, snumb[0:1, :w], st["rden"])
                    # ACT's DGE queue: an SP-issued DMA would park the SP
                    # sequencer on pred's semaphore and stall the x stream
                    nc.scalar.dma_start(out=out_d[OFFS[c]:OFFS[c] + w],
                                        in_=pred)

                return [A, B, C, D]

            # ---- block pipeline: stream block c+1 while post(c) runs.
            # post(c-1)'s phases are interleaved between block c's sub-DMA
            # accumulation groups so the chain's cross-engine stalls never
            # block the accumulation matmuls in the in-order PE stream.
            phases = []
            for c, w in enumerate(WIDTHS):
                hps = htp.tile([GE, WMAX], F32, tag="hps", name=f"hps{c}")
                base = 2 * TD * OFFS[c]

                def accum(hi_ap, lo_ap, k, p=128):
                    """3-pass bf16 split-precision accumulation for K-tile k."""
                    nc.tensor.matmul(hps[:, :w], lhsT=whi(k, p), rhs=hi_ap,
                                     start=(k == 0), stop=False)
                    nc.tensor.matmul(hps[:, :w], lhsT=wlo(k, p), rhs=hi_ap,
                                     start=False, stop=False)
                    nc.tensor.matmul(hps[:, :w], lhsT=whi(k, p), rhs=lo_ap,
                                     start=False, stop=(k == KT))

                k0 = 0
                for si, nk in enumerate(ksubs_for(w, last=(c == len(WIDTHS) - 1))):
                    if c == 0:
                        nwr = (KT + 1 - k0) if k0 + nk >= KT else nk
                        nc.sync.dma_start(out=wr_sb[:, k0:k0 + nwr, :],
                                          in_=wr_r[:, k0:k0 + nwr, :])
                    xs = xts.tile([128, 5120], BF16, tag="xt")
                    xv = xs[:, :nk * 2 * w].rearrange("p (k q m) -> p k q m",
                                                      k=nk, q=2)
                    src = xt_d[base + k0 * 256 * w:
                               base + (k0 + nk) * 256 * w]
                    nc.sync.dma_start(
                        out=xv, in_=src.rearrange("(k p q m) -> p k q m",
                                                  p=128, q=2, m=w))
                    for t in range(nk):
                        accum(xv[:, t, 0, :], xv[:, t, 1, :], k0 + t)
                    k0 += nk
                    if c == 0 and si == 0:
                        nc.sync.dma_start(out=mats_sb, in_=mats_d)
                        nc.sync.dma_start(out=bias_sb, in_=bias_d)
                    if si >= 1 and phases:
                        phases.pop(0)()
                # 8-row contraction remainder (rows 74*128 .. TD)
                xs8 = xts.tile([128, 5120], BF16, tag="xt", name=f"x8_{c}")
                xv8 = xs8[0:KREM, :2 * w].rearrange("p (q m) -> p q m", q=2)
                src8 = xt_d[base + KT * 256 * w:base + KT * 256 * w + KREM * 2 * w]
                nc.sync.dma_start(
                    out=xv8, in_=src8.rearrange("(p q m) -> p q m",
                                                p=KREM, q=2, m=w))
                accum(xv8[:, 0, :], xv8[:, 1, :], KT, p=KREM)
                while phases:
                    phases.pop(0)()
                phases = post_phases(c, hps, tail=(c == len(WIDTHS) - 1))
            for ph in phases:
                ph()

    nc.compile()
    return nc


_NC_CACHE = None
LAST_RESULTS = None


def kernel(x, Wr, br, We, be, Wq, bq, Wk, bk, Wv, bv, Wo, bo):
    global _NC_CACHE, LAST_RESULTS
    f32 = np.float32
    x = np.asarray(x, f32)

    wr_pack, mats_packed, biasp = build_consts(
        Wr, br, We, be, Wq, bq, Wk, bk, Wv, bv, Wo, bo)

    if _NC_CACHE is None:
        _NC_CACHE = build_kernel()
    nc = _NC_CACHE

    in_maps = []
    for core in range(NCORES):
        xs = x[core * NSH:(core + 1) * NSH].reshape(NSH, TD)
        in_maps.append({"xt": pack_x_shard(xs),
                        "wr": wr_pack.reshape(TDP, 2 * GE),
                        "mats": mats_packed, "bias": biasp})

    res = run_bass_kernel_spmd(nc, in_maps, list(range(NCORES)))
    LAST_RESULTS = res
    out = np.concatenate([res.results[core]["out"].reshape(NSH)
                          for core in range(NCORES)])
    return out.astype(f32)


# revision 33
# speedup vs baseline: 1.0033x; 1.0012x over previous
"""Trainium2 Bass kernel for nn_MIGAModel (moe_routing).

Strategy (pure data parallel over the stock axis N, 8 cores):
 - The x stream (94.8MB/core, ~264us at 360GB/s) is the roofline;
   everything else hides under it.  x ships as bf16 (hi, lo) plane
   pairs (same bytes as fp32, exact to ~2^-18), pre-transposed and
   column-blocked on host so each block is contiguous in DRAM.
 - Router h = Whi.xhi + Wlo.xhi + Whi.xlo: three bf16 matmuls per
   K-tile at 1 PE cycle/row (vs 4 for fp32) accumulate in fp32 PSUM.
   h error ~3e-6 -> zero top-2 flips (gating ranking is the
   precision-critical part: plain bf16/fp16 routing fails the 2e-2
   gate; plain fp32 matmuls would make PE the bottleneck; fp32r needs
   pre-rounded inputs on real HW, i.e. is not fp32-accurate).
 - Column-block pipeline: while block c+1's x streams, block c is
   post-processed; trailing blocks are narrower so the only exposed
   post-chain (the last block's) is short.
 - Host algebra folds the expert layer into Q/K/V (eo never exists),
   all q/k biases into one score matmul on h (MSCH) + a constant in
   the exp-activation bias, and the v biases into one matmul on the
   score numerators (MBV).  The value path runs in fp16 (error
   ~1e-3, gating unaffected; HW rel err ~1.25e-2 is dominated by a
   couple of top-2 near-tie flips from the router's ~3e-6 h-noise).
 - Gating and the final weighted combine use Pool-engine
   partition_all_reduce (max/add across the 128 expert partitions):
   no transposes or broadcast matmuls; the reduces overlap the DVE
   compare/select and value-path ops (real HW restricts Pool to the
   reduce family, DVE to one PSUM operand per op, and has no DVE
   divide -- hence the ACT-engine drains and recip+mul).
"""
import sys
import numpy as np
import ml_dtypes

for _p in ("/opt/trn_rl_repo",):
    if _p not in sys.path:
        sys.path.insert(0, _p)

import concourse.bass as bass
import concourse.tile as tile
from concourse import bacc, mybir, bass_isa
from concourse.bass_utils import run_bass_kernel_spmd

F32 = mybir.dt.float32
BF16 = mybir.dt.bfloat16
F16 = mybir.dt.float16
NPBF = ml_dtypes.bfloat16

N, T, D = 20000, 60, 158
TD = T * D                      # 9480
G, E, H, DH, GE = 8, 16, 4, 4, 128
NCORES = 8
NSH = N // NCORES               # 2500 rows per core
KT = TD // 128                  # 74 full K-tiles
KREM = TD - KT * 128            # 8 remainder contraction rows
TDP = (KT + 1) * 128            # wr ships padded to 75 tiles

# column blocks: sum = NSH; >=256 keeps DMA lines >=1KB; narrow
# trailing blocks shorten the exposed final post-chain
WIDTHS = [512, 512, 512, 452, 256, 128, 128]
assert sum(WIDTHS) == NSH and all(128 <= w <= 512 for w in WIDTHS)
WMAX = max(WIDTHS)
OFFS = [sum(WIDTHS[:i]) for i in range(len(WIDTHS))]


def ksubs_for(w, last=False):
    """K-tiles per sub-DMA over the 74 full tiles: ~10KB SBUF tiles.
    The final block tapers so almost no accumulation trails the stream."""
    nk = max(1, 2560 // w)
    ks = [nk] * (KT // nk)
    if KT % nk:
        ks.append(KT % nk)
    if last and ks[-1] > 4:
        ks[-1:] = [ks[-1] - 4, 2, 2]
    return ks


# packed matrix indices (each a [128,128] block in the "mats" input)
M_AQ = 0
M_AK0, M_AV0 = 1, 5             # 4 each
M_MS0 = 9                       # 4
M_MSCH = 13
M_MDEN = 14
M_MBV = 15
M_MER0 = 16                     # 4
M_AO = 20
NMATS = 21

# bias pack columns
B_BO, B_BR, B_SC = 0, 1, 2
NBIAS = 4


def bf16pair(a):
    """Split fp32 array into (hi, lo) bf16 planes, hi+lo ~= a to 2^-18."""
    a = np.asarray(a, np.float32)
    hi = a.astype(NPBF)
    lo = (a - hi.astype(np.float32)).astype(NPBF)
    return hi, lo


def build_consts(Wr, br, We, be, Wq, bq, Wk, bk, Wv, bv, Wo, bo):
    """Host-side packed constants. Returns (wr_pack, mats, biasp)."""
    f32 = np.float32
    Wr = np.asarray(Wr, f32)
    br = np.asarray(br, f32)
    We = np.asarray(We, f32)
    be = np.asarray(be, f32)
    Wq = np.asarray(Wq, f32)
    bq = np.asarray(bq, f32)
    Wk = np.asarray(Wk, f32)
    bk = np.asarray(bk, f32)
    Wv = np.asarray(Wv, f32)
    bv = np.asarray(bv, f32)
    Wo = np.asarray(Wo, f32)
    bo = np.asarray(bo, f32)

    # router weight: [TDP, 2, GE] bf16 (hi, lo planes), K-padded
    wr_pad = np.zeros((TDP, GE), f32)
    wr_pad[:TD] = Wr
    whi, wlo = bf16pair(wr_pad)
    wr_pack = np.zeros((TDP, 2, GE), NPBF)
    wr_pack[:, 0, :] = whi
    wr_pack[:, 1, :] = wlo

    mats = np.zeros((NMATS, GE, GE), f32)
    biasp = np.zeros((GE, NBIAS), f32)

    wet = np.transpose(We, (2, 0, 1)).reshape(GE, GE)
    be_vec = be.reshape(GE)
    biasp[:, B_BO] = bo.reshape(GE)
    biasp[:, B_BR] = br

    aq = np.zeros((GE, GE), f32)
    ak = np.zeros((DH, GE, GE), f32)
    av = np.zeros((DH, GE, GE), f32)
    bqp = np.zeros(GE, f32)
    bkp = np.zeros((DH, GE), f32)
    bvp = np.zeros((DH, GE), f32)
    ms = np.zeros((DH, GE, GE), f32)
    d_ = np.arange(DH)
    for g in range(G):
        for h in range(H):
            for d in range(DH):
                p = d * 32 + g * 4 + h
                aq[g * 16:(g + 1) * 16, p] = Wq[g, h * 4 + d, :]
                bqp[p] = bq[g, h * 4 + d]
            for e in range(DH):
                for p in d_ * 32 + g * 4 + h:
                    ak[e, g * 16:(g + 1) * 16, p] = Wk[g, h * 4 + e, :]
                    av[e, g * 16:(g + 1) * 16, p] = Wv[g, h * 4 + e, :]
                    bkp[e, p] = bk[g, h * 4 + e]
                    bvp[e, p] = bv[g, h * 4 + e]
    # expert-layer fold: y = A^T(WET^T h + be) + b -> (WET@A)^T h + (A^T be + b)
    mats[M_AQ] = wet @ aq
    bqp += aq.T @ be_vec
    for e in range(DH):
        mats[M_AK0 + e] = wet @ ak[e]
        mats[M_AV0 + e] = wet @ av[e]
        bkp[e] += ak[e].T @ be_vec
        bvp[e] += av[e].T @ be_vec

    for e in range(DH):
        for d in range(DH):
            for g in range(G):
                for h in range(H):
                    ms[e, d * 32 + g * 4 + h, e * 32 + d * 8 + g] = 1.0
                    mats[M_MDEN, e * 32 + d * 8 + g, d * 32 + g * 4 + h] = 1.0
                    mats[M_MER0 + e, e * 32 + d * 8 + g, d * 32 + g * 4 + h] = 1.0
        mats[M_MS0 + e] = ms[e]
    # q/k bias folds: scores = Sum_e MS_e^T(q*k_e) + MSCH^T h + const
    mats[M_MSCH] = mats[M_AQ] @ sum(bkp[e][:, None] * ms[e] for e in range(DH)) \
        + sum(mats[M_AK0 + e] @ (bqp[:, None] * ms[e]) for e in range(DH))
    sc_const = sum(ms[e].T @ (bqp * bkp[e]) for e in range(DH))
    biasp[:, B_SC] = 0.5 * sc_const
    # v bias fold: att_num = Sum_e er_e*v_e + MBV^T es
    mats[M_MBV] = sum(mats[M_MER0 + e] * bvp[e][None, :] for e in range(DH))
    for g in range(G):
        for f in range(E):
            for h in range(H):
                for d in range(DH):
                    mats[M_AO, d * 32 + g * 4 + h, g * 16 + f] = Wo[g, f, h * 4 + d]

    mats_packed = np.ascontiguousarray(
        np.transpose(mats, (1, 0, 2)).reshape(GE, NMATS * GE)).astype(np.float16)
    return wr_pack, mats_packed, biasp


def pack_x_shard(xs):
    """[NSH, TD] fp32 -> flat bf16 (block, ktile, part, plane, col) stream."""
    xt = np.zeros(TD * NSH * 2, NPBF)
    for c, w in enumerate(WIDTHS):
        hi, lo = bf16pair(xs[OFFS[c]:OFFS[c] + w].T)   # [TD, w] each
        blk = np.empty((TD, 2, w), NPBF)
        blk[:, 0, :] = hi
        blk[:, 1, :] = lo
        o = 2 * TD * OFFS[c]
        xt[o:o + 2 * TD * w] = blk.ravel()
    return xt


def build_kernel():
    """Trace the Bass/Tile kernel; returns the compiled Bacc."""
    nc = bacc.Bacc("TRN2", target_bir_lowering=False, debug=False,
                   num_devices=NCORES)

    xt_d = nc.dram_tensor("xt", [TD * NSH * 2], BF16, kind="ExternalInput").ap()
    wr_d = nc.dram_tensor("wr", [TDP, 2 * GE], BF16, kind="ExternalInput").ap()
    mats_d = nc.dram_tensor("mats", [GE, NMATS * GE], F16, kind="ExternalInput").ap()
    bias_d = nc.dram_tensor("bias", [GE, NBIAS], F32, kind="ExternalInput").ap()
    out_d = nc.dram_tensor("out", [NSH], F32, kind="ExternalOutput").ap()

    RMAX = bass_isa.ReduceOp.max
    RADD = bass_isa.ReduceOp.add

    with tile.TileContext(nc) as tc:
        with (
            nc.allow_low_precision(reason="bf16 value path is validated to "
                                   "2.7e-3 rel err; gating stays fp32"),
            tc.tile_pool(name="consts", bufs=1) as consts,
            tc.tile_pool(name="xts", bufs=6) as xts,
            tc.tile_pool(name="work", bufs=2) as work,
            tc.tile_pool(name="ps", bufs=6, space="PSUM") as ptp,
            tc.tile_pool(name="hp", bufs=2, space="PSUM") as htp,
        ):
            # ---- constants ----
            # wr streams in per-K-group pieces interleaved with block 0's x
            # subs (a single up-front 4.9MB wr DMA would delay the first
            # router matmul by ~14us); mats/bias follow block 0's first sub.
            wr_sb = consts.tile([128, KT + 1, 2 * GE], BF16, tag="wr")
            wr_r = wr_d.rearrange("(c p) m -> p c m", p=128)
            mats_sb = consts.tile([GE, NMATS * GE], F16, tag="mats")
            bias_sb = consts.tile([GE, NBIAS], F32, tag="bias")

            def mat(i):
                return mats_sb[:, i * GE:(i + 1) * GE]

            def whi(k, p=128):
                return wr_sb[0:p, k, 0:GE]

            def wlo(k, p=128):
                return wr_sb[0:p, k, GE:2 * GE]

            def bcol(i):
                return bias_sb[:, i:i + 1]

            def wtile(tag, c, dt=F32, p=GE):
                return work.tile([p, WMAX], dt, tag=tag, name=f"{tag}{c}")

            def post_phases(c, hps, tail=False):
                """Post-processing split into 4 phases so the PE stream can
                interleave the next block's accumulation matmuls between the
                chain's cross-engine stalls.  For the exposed final block
                (tail=True) the value path is emitted before the gating ops,
                since there the chain latency itself is the kernel tail."""
                w = WIDTHS[c]
                st = {}

                def gating(h_sb, eh):
                    # exact fp32 top-2 gating: reduces on Pool, the rest DVE
                    mx1b = wtile("mx1b", c)[:, :w]
                    nc.gpsimd.partition_all_reduce(mx1b, h_sb, channels=GE,
                                                   reduce_op=RMAX)
                    eqm = wtile("eqm", c)[:, :w]
                    nc.vector.tensor_tensor(eqm, h_sb, mx1b,
                                            op=mybir.AluOpType.is_ge)
                    hm = wtile("hm", c)[:, :w]
                    nc.vector.scalar_tensor_tensor(
                        hm, in0=eqm, scalar=-1e30, in1=h_sb,
                        op0=mybir.AluOpType.mult, op1=mybir.AluOpType.add)
                    mx2b = wtile("mx2b", c)[:, :w]
                    nc.gpsimd.partition_all_reduce(mx2b, hm, channels=GE,
                                                   reduce_op=RMAX)
                    m1 = wtile("m1", c)[:, :w]
                    nc.vector.tensor_tensor(m1, h_sb, mx2b,
                                            op=mybir.AluOpType.is_ge)
                    nc.vector.tensor_mul(m1, m1, eh)
                    sdenb = wtile("sdenb", c)[:, :w]
                    nc.gpsimd.partition_all_reduce(sdenb, m1, channels=GE,
                                                   reduce_op=RADD)
                    rden = wtile("rden", c, p=1)[:, :w]
                    nc.vector.reciprocal(rden, sdenb[0:1, :w])
                    st.update(m1=m1, sdenb=sdenb, rden=rden)

                def heads(hb):
                    # all h-fed matmuls up front: PE never waits here
                    q_ps = ptp.tile([GE, WMAX], F32, tag="pt", name=f"q{c}")
                    nc.tensor.matmul(q_ps[:, :w], lhsT=mat(M_AQ), rhs=hb,
                                     start=True, stop=True)
                    k_ps = [ptp.tile([GE, WMAX], F32, tag="pt", name=f"k{c}_{e}")
                            for e in range(DH)]
                    for e in range(DH):
                        nc.tensor.matmul(k_ps[e][:, :w], lhsT=mat(M_AK0 + e),
                                         rhs=hb, start=True, stop=True)
                    # HW: DVE reads at most one PSUM operand, so q drains
                    # through the (mostly idle) ACT engine first
                    qb = wtile("qb", c, F16)[:, :w]
                    nc.scalar.activation(qb, q_ps[:, :w],
                                         mybir.ActivationFunctionType.Identity)
                    pes = []
                    for e in range(DH):
                        pe_sb = wtile(f"pe{e % 2}", c, F16)[:, :w]
                        nc.vector.tensor_mul(pe_sb, qb, k_ps[e][:, :w])
                        pes.append(pe_sb)
                    if tail:
                        # pull the v matmuls + drains off the exposed C chain
                        vbs = []
                        for e in range(DH):
                            v_ps = ptp.tile([GE, WMAX], F32, tag="pt",
                                            name=f"v{c}_{e}")
                            nc.tensor.matmul(v_ps[:, :w], lhsT=mat(M_AV0 + e),
                                             rhs=hb, start=True, stop=True)
                            vb = wtile(f"vb{e % 2}", c, F16)[:, :w]
                            nc.scalar.activation(
                                vb, v_ps[:, :w],
                                mybir.ActivationFunctionType.Identity)
                            vbs.append(vb)
                        st["vbs"] = vbs
                    st.update(hb=hb, pes=pes)

                def A():
                    h_sb = wtile("h", c)[:, :w]
                    nc.scalar.activation(h_sb, hps[:, :w],
                                         mybir.ActivationFunctionType.Identity,
                                         bias=bcol(B_BR), scale=1.0)
                    if tail:
                        hb = wtile("hb", c, F16)[:, :w]
                        nc.scalar.activation(hb, hps[:, :w],
                                             mybir.ActivationFunctionType.Identity,
                                             bias=bcol(B_BR), scale=1.0)
                        eh = wtile("eh", c)[:, :w]
                        nc.scalar.activation(eh, hps[:, :w],
                                             mybir.ActivationFunctionType.Exp,
                                             bias=bcol(B_BR), scale=1.0)
                        heads(hb)
                        gating(h_sb, eh)
                    else:
                        eh = wtile("eh", c)[:, :w]
                        nc.scalar.activation(eh, hps[:, :w],
                                             mybir.ActivationFunctionType.Exp,
                                             bias=bcol(B_BR), scale=1.0)
                        hb = wtile("hb", c, F16)[:, :w]
                        nc.scalar.activation(hb, hps[:, :w],
                                             mybir.ActivationFunctionType.Identity,
                                             bias=bcol(B_BR), scale=1.0)
                        gating(h_sb, eh)
                        heads(hb)

                def B():
                    sc_ps = ptp.tile([GE, WMAX], F32, tag="pt", name=f"sc{c}")
                    nc.tensor.matmul(sc_ps[:, :w], lhsT=mat(M_MSCH),
                                     rhs=st["hb"], start=True, stop=False)
                    for e in range(DH):
                        nc.tensor.matmul(sc_ps[:, :w], lhsT=mat(M_MS0 + e),
                                         rhs=st["pes"][e],
                                         start=False, stop=(e == DH - 1))
                    es_sb = wtile("es", c, F16)[:, :w]
                    nc.scalar.activation(es_sb, sc_ps[:, :w],
                                         mybir.ActivationFunctionType.Exp,
                                         bias=bcol(B_SC), scale=0.5)
                    st["es"] = es_sb

                def C():
                    es_sb = st["es"]
                    prs = []
                    for e in range(DH):
                        er_ps = ptp.tile([GE, WMAX], F32, tag="pt",
                                         name=f"er{c}_{e}")
                        nc.tensor.matmul(er_ps[:, :w], lhsT=mat(M_MER0 + e),
                                         rhs=es_sb, start=True, stop=True)
                        if tail:
                            vb = st["vbs"][e]
                        else:
                            v_ps = ptp.tile([GE, WMAX], F32, tag="pt",
                                            name=f"v{c}_{e}")
                            nc.tensor.matmul(v_ps[:, :w], lhsT=mat(M_AV0 + e),
                                             rhs=st["hb"], start=True, stop=True)
                            vb = wtile(f"vb{e % 2}", c, F16)[:, :w]
                            nc.scalar.activation(
                                vb, v_ps[:, :w],
                                mybir.ActivationFunctionType.Identity)
                        pr = wtile(f"pr{e % 2}", c, F16)[:, :w]
                        nc.vector.tensor_mul(pr, er_ps[:, :w], vb)
                        prs.append(pr)
                    bv_ps = ptp.tile([GE, WMAX], F32, tag="pt", name=f"bv{c}")
                    nc.tensor.matmul(bv_ps[:, :w], lhsT=mat(M_MBV), rhs=es_sb,
                                     start=True, stop=True)
                    den_ps = ptp.tile([GE, WMAX], F32, tag="pt", name=f"den{c}")
                    nc.tensor.matmul(den_ps[:, :w], lhsT=mat(M_MDEN),
                                     rhs=es_sb, start=True, stop=True)
                    t01 = wtile("t01", c, F16)[:, :w]
                    nc.vector.tensor_add(t01, prs[0], prs[1])
                    t23 = wtile("t23", c, F16)[:, :w]
                    nc.vector.tensor_add(t23, prs[2], prs[3])
                    drec = wtile("drec", c, F16)[:, :w]
                    nc.vector.reciprocal(drec, den_ps[:, :w])
                    att = wtile("att", c, F16)[:, :w]
                    nc.vector.tensor_add(att, t01, t23)
                    nc.vector.tensor_add(att, att, bv_ps[:, :w])
                    nc.vector.tensor_mul(att, att, drec)
                    st["att"] = att

                def D():
                    ao_ps = ptp.tile([GE, WMAX], F32, tag="pt", name=f"ao{c}")
                    nc.tensor.matmul(ao_ps[:, :w], lhsT=mat(M_AO),
                                     rhs=st["att"], start=True, stop=True)
                    aout = wtile("aout", c)[:, :w]
                    nc.vector.tensor_scalar_add(aout, ao_ps[:, :w], bcol(B_BO))
                    num = wtile("num", c)[:, :w]
                    nc.vector.tensor_mul(num, st["m1"], aout)
                    snumb = wtile("snumb", c)[:, :w]
                    nc.gpsimd.partition_all_reduce(snumb, num, channels=GE,
                                                   reduce_op=RADD)
                    pred = wtile("pred", c, p=1)[:, :w]
                    nc.vector.tensor_mul(pred, snumb[0:1, :w], st["rden"])
                    # ACT's DGE queue: an SP-issued DMA would park the SP
                    # sequencer on pred's semaphore and stall the x stream
                    nc.scalar.dma_start(out=out_d[OFFS[c]:OFFS[c] + w],
                                        in_=pred)

                return [A, B, C, D]

            # ---- block pipeline: stream block c+1 while post(c) runs.
            # post(c-1)'s phases are interleaved between block c's sub-DMA
            # accumulation groups so the chain's cross-engine stalls never
            # block the accumulation matmuls in the in-order PE stream.
            phases = []
            for c, w in enumerate(WIDTHS):
                hps = htp.tile([GE, WMAX], F32, tag="hps", name=f"hps{c}")
                base = 2 * TD * OFFS[c]

                def accum(hi_ap, lo_ap, k, p=128):
                    """3-pass bf16 split-precision accumulation for K-tile k."""
                    nc.tensor.matmul(hps[:, :w], lhsT=whi(k, p), rhs=hi_ap,
                                     start=(k == 0), stop=False)
                    nc.tensor.matmul(hps[:, :w], lhsT=wlo(k, p), rhs=hi_ap,
                                     start=False, stop=False)
                    nc.tensor.matmul(hps[:, :w], lhsT=whi(k, p), rhs=lo_ap,
                                     start=False, stop=(k == KT))

                k0 = 0
                for si, nk in enumerate(ksubs_for(w, last=(c == len(WIDTHS) - 1))):
                    if c == 0:
                        nwr = (KT + 1 - k0) if k0 + nk >= KT else nk
                        nc.sync.dma_start(out=wr_sb[:, k0:k0 + nwr, :],
                                          in_=wr_r[:, k0:k0 + nwr, :])
                    xs = xts.tile([128, 5120], BF16, tag="xt")
                    xv = xs[:, :nk * 2 * w].rearrange("p (k q m) -> p k q m",
                                                      k=nk, q=2)
                    src = xt_d[base + k0 * 256 * w:
                               base + (k0 + nk) * 256 * w]
                    nc.sync.dma_start(
                        out=xv, in_=src.rearrange("(k p q m) -> p k q m",
                                                  p=128, q=2, m=w))
                    for t in range(nk):
                        accum(xv[:, t, 0, :], xv[:, t, 1, :], k0 + t)
                    k0 += nk
                    if c == 0 and si == 0:
                        nc.sync.dma_start(out=mats_sb, in_=mats_d)
                        nc.sync.dma_start(out=bias_sb, in_=bias_d)
                    if si >= 1 and phases:
                        phases.pop(0)()
                # 8-row contraction remainder (rows 74*128 .. TD)
                xs8 = xts.tile([128, 5120], BF16, tag="xt", name=f"x8_{c}")
                xv8 = xs8[0:KREM, :2 * w].rearrange("p (q m) -> p q m", q=2)
                src8 = xt_d[base + KT * 256 * w:base + KT * 256 * w + KREM * 2 * w]
                nc.sync.dma_start(
                    out=xv8, in_=src8.rearrange("(p q m) -> p q m",
                                                p=KREM, q=2, m=w))
                accum(xv8[:, 0, :], xv8[:, 1, :], KT, p=KREM)
                while phases:
                    phases.pop(0)()
                phases = post_phases(c, hps, tail=(c == len(WIDTHS) - 1))
            for ph in phases:
                ph()

    nc.compile()
    return nc


_NC_CACHE = None
LAST_RESULTS = None


def kernel(x, Wr, br, We, be, Wq, bq, Wk, bk, Wv, bv, Wo, bo):
    global _NC_CACHE, LAST_RESULTS
    f32 = np.float32
    x = np.asarray(x, f32)

    wr_pack, mats_packed, biasp = build_consts(
        Wr, br, We, be, Wq, bq, Wk, bk, Wv, bv, Wo, bo)

    if _NC_CACHE is None:
        _NC_CACHE = build_kernel()
    nc = _NC_CACHE

    in_maps = []
    for core in range(NCORES):
        xs = x[core * NSH:(core + 1) * NSH].reshape(NSH, TD)
        in_maps.append({"xt": pack_x_shard(xs),
                        "wr": wr_pack.reshape(TDP, 2 * GE),
                        "mats": mats_packed, "bias": biasp})

    res = run_bass_kernel_spmd(nc, in_maps, list(range(NCORES)))
    LAST_RESULTS = res
    out = np.concatenate([res.results[core]["out"].reshape(NSH)
                          for core in range(NCORES)])
    return out.astype(f32)
